# revision 5
# baseline (speedup 1.0000x reference)
"""GroupLinear Trainium2 kernel.

out[b, g, o] = sum_i x[b, i] * W[g, o, i] + b[g, o]
  x: (4096, 1024) f32, W: (16, 1024, 1024) f32, b: (16, 1024) f32
  out: (4096, 16, 1024) f32

Sharding: groups across the 8 cores (2 groups/core), x replicated.

Compute strategy: fp8(e4m3) hi/lo split with DoubleRow matmuls. Host
pre-transposes and splits x*sx and W*sw into hi + lo fp8 parts; the device
accumulates the three cross terms hi*hi + lo*hi + hi*lo (the lo*lo term is
~1e-5 relative, dropped) into one fp32 PSUM chain per output chunk.
DoubleRow processes two 128-row k-planes per instruction at 2 MACs/PE/cycle,
so the 3-term scheme runs at 1.5 cycles per fp32-equivalent column vs 2 for
bf16 — ~164us of PE time instead of ~221us. Rel err ~2e-3 (vs 2e-2 gate).

Scheduling: leading batch tiles run pair-interleaved so the PE has work
while W streams in (hi terms are ordered before lo terms there, matching
the W-hi-before-W-lo DMA order); bias (pre-scaled by sx*sw) is fused into
the PSUM evacuation; the host divides the bf16 output by sx*sw.
"""

import sys
import types

sys.path.insert(0, "/opt/trn_rl_repo")

# Provide antenv.axon_hooks (NTFF profile hook registry) if the installed
# antenv lacks it — the axon boot registers its profiling hook here, and
# concourse.bass_utils reads it back when trace=True. Must exist before the
# first jax/axon backend init.
try:
    from antenv import axon_hooks as _axon_hooks  # noqa: F401
except ImportError:
    _m = types.ModuleType("antenv.axon_hooks")
    _m._hook = None

    def _set_hook(hook, _m=_m):
        _m._hook = hook

    def _get_hook(_m=_m):
        return _m._hook

    _m.set_axon_ntff_profile_hook = _set_hook
    _m.get_axon_ntff_profile_hook = _get_hook
    sys.modules["antenv.axon_hooks"] = _m
    try:
        import antenv

        antenv.axon_hooks = _m
    except ImportError:
        pass

from contextlib import ExitStack

import ml_dtypes
import numpy as np

import concourse.bass as bass
import concourse.mybir as mybir
import concourse.tile as tile
from concourse import bacc
from concourse.bass_utils import run_bass_kernel_spmd

F32 = mybir.dt.float32
BF16 = mybir.dt.bfloat16
FP8 = mybir.dt.float8e4
FP8NP = ml_dtypes.float8_e4m3
DR = mybir.MatmulPerfMode.DoubleRow

BATCH, D_IN, D_OUT, GROUPS, NCORES = 4096, 1024, 1024, 16, 8
GPC = GROUPS // NCORES  # groups per core
PAIR_TILES = 4          # leading batch tiles run as pair-interleaved chains
SX, SW = 16.0, 2048.0   # fp8 pre-scales (keep e4m3 max-240 headroom)
SCALE = SX * SW


def build_nc(batch=BATCH, d_in=D_IN, d_out=D_OUT, gpc=GPC):
    P = 128
    KT = d_in // P           # k-tiles (128-row planes) along contraction
    KT2 = KT // 2            # DoubleRow consumes plane pairs
    MT = batch // P          # batch tiles
    CW = 512                 # matmul moving free dim (1 psum bank fp32)
    NCH = gpc * d_out // CW  # output chunks per batch tile
    BQ = 512                 # batch columns per x-load chunk

    nc = bacc.Bacc("TRN2", target_bir_lowering=False, debug=False)
    # host-pretransposed hi/lo fp8 splits: x*[kt, p, b] = fp8(x[b, kt*128+p]*SX)
    xh = nc.dram_tensor("xh", [KT, P, batch], FP8, kind="ExternalInput").ap()
    xl = nc.dram_tensor("xl", [KT, P, batch], FP8, kind="ExternalInput").ap()
    # W*[g, kt, p, o] = fp8(W[g, o, kt*128+p]*SW)
    Wh = nc.dram_tensor("Wh", [gpc, KT, P, d_out], FP8, kind="ExternalInput").ap()
    Wl = nc.dram_tensor("Wl", [gpc, KT, P, d_out], FP8, kind="ExternalInput").ap()
    b = nc.dram_tensor("b", [gpc, d_out], F32, kind="ExternalInput").ap()
    out = nc.dram_tensor("out", [batch, gpc * d_out], BF16, kind="ExternalOutput").ap()

    with ExitStack() as ctx:
        tc = ctx.enter_context(tile.TileContext(nc))
        singles = ctx.enter_context(tc.tile_pool(name="singles", bufs=1))
        out_pool = ctx.enter_context(tc.tile_pool(name="outp", bufs=8))
        ps_mm = ctx.enter_context(tc.tile_pool(name="ps_mm", bufs=8, space="PSUM"))

        # bias (pre-scaled by SCALE on host) broadcast to all 128 partitions,
        # on the output queue so the input queue stays clear
        bias_sb = singles.tile([P, gpc * d_out], F32)
        b_bcast = bass.AP(
            tensor=b.tensor, offset=b.offset, ap=[[0, P], [1, gpc * d_out]]
        )
        nc.scalar.dma_start(out=bias_sb[:, :], in_=b_bcast)

        wh8 = singles.tile([P, KT, gpc * d_out], FP8)
        wl8 = singles.tile([P, KT, gpc * d_out], FP8)
        xh8 = singles.tile([P, KT, batch], FP8)
        xl8 = singles.tile([P, KT, batch], FP8)

        # Input DMA order mirrors warmup consumption: x q0-chunk + W-hi
        # kt-major, then W-lo, then remaining x chunks.
        for kt in range(KT):
            nc.sync.dma_start(out=xh8[:, kt, 0:BQ], in_=xh[kt, :, 0:BQ])
            nc.sync.dma_start(out=xl8[:, kt, 0:BQ], in_=xl[kt, :, 0:BQ])
            for g in range(gpc):
                nc.sync.dma_start(
                    out=wh8[:, kt, g * d_out : (g + 1) * d_out], in_=Wh[g, kt]
                )
        for kt in range(KT):
            for g in range(gpc):
                nc.sync.dma_start(
                    out=wl8[:, kt, g * d_out : (g + 1) * d_out], in_=Wl[g, kt]
                )
        for q in range(1, batch // BQ):
            for kt in range(KT):
                nc.sync.dma_start(
                    out=xh8[:, kt, q * BQ : (q + 1) * BQ],
                    in_=xh[kt, :, q * BQ : (q + 1) * BQ],
                )
                nc.sync.dma_start(
                    out=xl8[:, kt, q * BQ : (q + 1) * BQ],
                    in_=xl[kt, :, q * BQ : (q + 1) * BQ],
                )

        TERMS = ((xh8, wh8), (xl8, wh8), (xh8, wl8))

        def alloc_banks(m):
            return [
                ps_mm.tile([P, CW], F32, tag="ps_mm", name=f"ps_mm_{m}_{c}")
                for c in range(NCH)
            ]

        def mm_step(pss, m, t, term, start, stop):
            xsrc, wsrc = TERMS[term]
            lhsT = xsrc[:, 2 * t : 2 * t + 2, m * P : (m + 1) * P]
            for c in range(NCH):
                nc.tensor.matmul(
                    pss[c][:, :],
                    lhsT,
                    wsrc[:, 2 * t : 2 * t + 2, c * CW : (c + 1) * CW],
                    start=start,
                    stop=stop,
                    perf_mode=DR,
                )

        def evac(pss, m):
            # bias add + bf16 cast on DVE; per-chunk output DMA
            for c in range(NCH):
                o_sb = out_pool.tile([P, CW], BF16, tag="outp")
                nc.vector.tensor_add(
                    out=o_sb[:, :],
                    in0=pss[c][:, :],
                    in1=bias_sb[:, c * CW : (c + 1) * CW],
                )
                nc.scalar.dma_start(
                    out=out[m * P : (m + 1) * P, c * CW : (c + 1) * CW],
                    in_=o_sb[:, :],
                )

        # warmup: pair-interleaved chains, hi terms first (W-lo still in
        # flight while these run)
        for j in range(PAIR_TILES // 2):
            ms = (2 * j, 2 * j + 1)
            pss = {m: alloc_banks(m) for m in ms}
            for t in range(KT2):
                for m in ms:
                    for term in (0, 1):
                        mm_step(pss[m], m, t, term, start=(t == 0 and term == 0),
                                stop=False)
            for t in range(KT2):
                for m in ms:
                    mm_step(pss[m], m, t, 2, start=False, stop=(t == KT2 - 1))
            for m in ms:
                evac(pss[m], m)

        # steady state: per-tile chains
        for m in range(PAIR_TILES, MT):
            pss = alloc_banks(m)
            for t in range(KT2):
                for term in range(3):
                    mm_step(pss, m, t, term, start=(t == 0 and term == 0),
                            stop=(t == KT2 - 1 and term == 2))
            evac(pss, m)

    nc.finalize()
    return nc


_NC_CACHE = {}


def _get_nc(key=(BATCH, D_IN, D_OUT, GPC)):
    if key not in _NC_CACHE:
        _NC_CACHE[key] = build_nc(*key)
    return _NC_CACHE[key]


def _split8(a, s):
    hi = (a * s).astype(FP8NP)
    lo = ((a * s) - hi.astype(np.float32)).astype(FP8NP)
    return hi, lo


def _run(inputs, trace=False):
    x = np.asarray(inputs["x"], dtype=np.float32)
    W = np.asarray(inputs["W"], dtype=np.float32)
    b = np.asarray(inputs["b"], dtype=np.float32)

    KT = D_IN // 128
    x_hi, x_lo = _split8(x, SX)
    # x*[kt, p, b] = fp8(x[b, kt*128+p]*SX)
    xh = np.ascontiguousarray(x_hi.T).reshape(KT, 128, BATCH)
    xl = np.ascontiguousarray(x_lo.T).reshape(KT, 128, BATCH)
    W_hi, W_lo = _split8(W, SW)

    nc = _get_nc()
    in_maps = []
    for c in range(NCORES):
        # W*[g, kt, p, o] = fp8(W[c*GPC+g, o, kt*128+p]*SW)
        Whc = np.ascontiguousarray(
            W_hi[c * GPC : (c + 1) * GPC].transpose(0, 2, 1)
        ).reshape(GPC, KT, 128, D_OUT)
        Wlc = np.ascontiguousarray(
            W_lo[c * GPC : (c + 1) * GPC].transpose(0, 2, 1)
        ).reshape(GPC, KT, 128, D_OUT)
        in_maps.append(
            {
                "xh": xh,
                "xl": xl,
                "Wh": Whc,
                "Wl": Wlc,
                "b": np.ascontiguousarray(b[c * GPC : (c + 1) * GPC]) * SCALE,
            }
        )
    res = run_bass_kernel_spmd(nc, in_maps, core_ids=list(range(NCORES)), trace=trace)
    shards = [r["out"] for r in res.results]
    full = np.concatenate(shards, axis=1).astype(np.float32) * (1.0 / SCALE)
    return full.reshape(BATCH, GROUPS, D_OUT), res


def kernel(**inputs):
    out, _ = _run(inputs, trace=False)
    return out


# revision 10
# speedup vs baseline: 1.4182x; 1.4182x over previous
"""GroupLinear Trainium2 kernel.

out[b, g, o] = sum_i x[b, i] * W[g, o, i] + b[g, o]
  x: (4096, 1024) f32, W: (16, 1024, 1024) f32, b: (16, 1024) f32
  out: (4096, 16, 1024) f32

Sharding: groups across the 8 cores (2 groups/core), x replicated.

Layout strategy: x and W are transposed + cast to bf16 on the host so the
contraction dim (i) lands on SBUF partitions with no on-device transposes.
The device kernel is then a pure back-to-back bf16 matmul stream (keeps the
PE p-state ramped to max clock), bias fused into the PSUM->SBUF evacuation,
bf16 output upcast on the host.

Scheduling: the first batch tiles run as pair-interleaved accumulation
chains (2 tiles x 4 chunks = 8 psum banks live) so the PE has ~2x work per
arriving W k-slab and never starves while W streams in; the bias broadcast
rides the output queue to keep the input queue dedicated to x/W.
"""

import sys
import types

sys.path.insert(0, "/opt/trn_rl_repo")

# Provide antenv.axon_hooks (NTFF profile hook registry) if the installed
# antenv lacks it — the axon boot registers its profiling hook here, and
# concourse.bass_utils reads it back when trace=True. Must exist before the
# first jax/axon backend init.
try:
    from antenv import axon_hooks as _axon_hooks  # noqa: F401
except ImportError:
    _m = types.ModuleType("antenv.axon_hooks")
    _m._hook = None

    def _set_hook(hook, _m=_m):
        _m._hook = hook

    def _get_hook(_m=_m):
        return _m._hook

    _m.set_axon_ntff_profile_hook = _set_hook
    _m.get_axon_ntff_profile_hook = _get_hook
    sys.modules["antenv.axon_hooks"] = _m
    try:
        import antenv

        antenv.axon_hooks = _m
    except ImportError:
        pass

from contextlib import ExitStack

import ml_dtypes
import numpy as np

import concourse.bass as bass
import concourse.mybir as mybir
import concourse.tile as tile
from concourse import bacc
from concourse.bass_utils import run_bass_kernel_spmd

F32 = mybir.dt.float32
BF16 = mybir.dt.bfloat16
BF16NP = ml_dtypes.bfloat16

BATCH, D_IN, D_OUT, GROUPS, NCORES = 4096, 1024, 1024, 16, 8
GPC = GROUPS // NCORES  # groups per core
PAIR_TILES = 4          # leading batch tiles run as pair-interleaved chains


def build_nc(batch=BATCH, d_in=D_IN, d_out=D_OUT, gpc=GPC):
    P = 128
    KT = d_in // P           # k-tiles along contraction
    MT = batch // P          # batch tiles
    CW = 512                 # matmul moving free dim (1 psum bank fp32)
    NCH = gpc * d_out // CW  # output chunks per batch tile
    BQ = 512                 # batch columns per x-load chunk

    nc = bacc.Bacc("TRN2", target_bir_lowering=False, debug=False)
    # host-pretransposed: xT[kt, p, b] = x[b, kt*128+p]
    xT = nc.dram_tensor("xT", [KT, P, batch], BF16, kind="ExternalInput").ap()
    # host-pretransposed: WT[g, kt, p, o] = W[g, o, kt*128+p]
    WT = nc.dram_tensor("WT", [gpc, KT, P, d_out], BF16, kind="ExternalInput").ap()
    b = nc.dram_tensor("b", [gpc, d_out], F32, kind="ExternalInput").ap()
    out = nc.dram_tensor("out", [batch, gpc * d_out], BF16, kind="ExternalOutput").ap()

    with ExitStack() as ctx:
        tc = ctx.enter_context(tile.TileContext(nc))
        singles = ctx.enter_context(tc.tile_pool(name="singles", bufs=1))
        out_pool = ctx.enter_context(tc.tile_pool(name="outp", bufs=8))
        ps_mm = ctx.enter_context(tc.tile_pool(name="ps_mm", bufs=8, space="PSUM"))

        # bias broadcast to all 128 partitions, on the output queue so the
        # input queue stays dedicated to the critical x/W stream
        bias_sb = singles.tile([P, gpc * d_out], F32)
        b_bcast = bass.AP(
            tensor=b.tensor, offset=b.offset, ap=[[0, P], [1, gpc * d_out]]
        )
        nc.scalar.dma_start(out=bias_sb[:, :], in_=b_bcast)

        wt = singles.tile([P, KT, gpc * d_out], BF16)
        xt = singles.tile([P, KT, batch], BF16)

        # Inputs ride two queues in parallel (both have a multi-us slow
        # start, so splitting halves the time to the first usable k-slab):
        # x chunks on sync, W kt-major on gpsimd.
        for kt in range(KT):
            nc.sync.dma_start(out=xt[:, kt, 0:BQ], in_=xT[kt, :, 0:BQ])
        for q in range(1, batch // BQ):
            for kt in range(KT):
                nc.sync.dma_start(
                    out=xt[:, kt, q * BQ : (q + 1) * BQ],
                    in_=xT[kt, :, q * BQ : (q + 1) * BQ],
                )
        for kt in range(KT):
            for g in range(gpc):
                nc.gpsimd.dma_start(
                    out=wt[:, kt, g * d_out : (g + 1) * d_out], in_=WT[g, kt]
                )

        # PE prewarm: dummy matmuls on a memset tile eat the DVFS ramp
        # (0.65/1.2 GHz p-states) while the first input slabs stream in, so
        # real matmuls start at full clock.
        scratch = singles.tile([P, CW], BF16)
        ps_warm = ps_mm.tile([P, CW], F32, tag="ps_mm", name="ps_warm")
        nc.vector.memset(scratch[:, :], 0.0)
        for _ in range(6):
            nc.tensor.matmul(
                ps_warm[:, :], scratch[:, 0:P], scratch[:, :], start=True, stop=True
            )

        def alloc_banks(m):
            return [
                ps_mm.tile([P, CW], F32, tag="ps_mm", name=f"ps_mm_{m}_{c}")
                for c in range(NCH)
            ]

        def chain_step(pss, m, kt):
            lhsT = xt[:, kt, m * P : (m + 1) * P]
            for c in range(NCH):
                nc.tensor.matmul(
                    pss[c][:, :],
                    lhsT,
                    wt[:, kt, c * CW : (c + 1) * CW],
                    start=(kt == 0),
                    stop=(kt == KT - 1),
                )

        def evac_chunk(ps, m, c):
            # bias add + bf16 cast on DVE (GpSimd cannot read PSUM), then
            # the chunk's output DMA
            o_sb = out_pool.tile([P, CW], BF16, tag="outp")
            nc.vector.tensor_add(
                out=o_sb[:, :],
                in0=ps[:, :],
                in1=bias_sb[:, c * CW : (c + 1) * CW],
            )
            nc.scalar.dma_start(
                out=out[m * P : (m + 1) * P, c * CW : (c + 1) * CW],
                in_=o_sb[:, :],
            )

        def evac(pss, m):
            for c in range(NCH):
                evac_chunk(pss[c], m, c)

        # warmup: pair-interleaved chains (8 psum banks live)
        for j in range(PAIR_TILES // 2):
            ms = (2 * j, 2 * j + 1)
            pss = {m: alloc_banks(m) for m in ms}
            for kt in range(KT):
                for m in ms:
                    chain_step(pss[m], m, kt)
            for m in ms:
                evac(pss[m], m)

        # steady state: per-tile chains (4 banks, short evac tail)
        for m in range(PAIR_TILES, MT - 1):
            pss = alloc_banks(m)
            for kt in range(KT):
                chain_step(pss, m, kt)
            evac(pss, m)

        # last tile runs chunk-major so each chunk's evacuation overlaps the
        # remaining chunks' matmuls; only the final chunk's evac trails
        m = MT - 1
        pss = alloc_banks(m)
        for c in range(NCH):
            for kt in range(KT):
                nc.tensor.matmul(
                    pss[c][:, :],
                    xt[:, kt, m * P : (m + 1) * P],
                    wt[:, kt, c * CW : (c + 1) * CW],
                    start=(kt == 0),
                    stop=(kt == KT - 1),
                )
            evac_chunk(pss[c], m, c)

    nc.finalize()
    return nc


_NC_CACHE = {}


def _get_nc(key=(BATCH, D_IN, D_OUT, GPC)):
    if key not in _NC_CACHE:
        _NC_CACHE[key] = build_nc(*key)
    return _NC_CACHE[key]


def _run(inputs, trace=False):
    x = np.asarray(inputs["x"], dtype=np.float32)
    W = np.asarray(inputs["W"], dtype=np.float32)
    b = np.asarray(inputs["b"], dtype=np.float32)

    KT = D_IN // 128
    # xT[kt, p, b] = x[b, kt*128+p]
    xT = np.ascontiguousarray(x.astype(BF16NP).T).reshape(KT, 128, BATCH)
    W_bf = W.astype(BF16NP)

    nc = _get_nc()
    in_maps = []
    for c in range(NCORES):
        # WT[g, kt, p, o] = W[c*GPC+g, o, kt*128+p]
        Wc = np.ascontiguousarray(
            W_bf[c * GPC : (c + 1) * GPC].transpose(0, 2, 1)
        ).reshape(GPC, KT, 128, D_OUT)
        in_maps.append(
            {
                "xT": xT,
                "WT": Wc,
                "b": np.ascontiguousarray(b[c * GPC : (c + 1) * GPC]),
            }
        )
    res = run_bass_kernel_spmd(nc, in_maps, core_ids=list(range(NCORES)), trace=trace)
    shards = [r["out"] for r in res.results]
    full = np.concatenate(shards, axis=1).astype(np.float32)
    return full.reshape(BATCH, GROUPS, D_OUT), res


def kernel(**inputs):
    out, _ = _run(inputs, trace=False)
    return out
